# revision 26
# baseline (speedup 1.0000x reference)
"""DeBERTa layer on 8 trn2 NeuronCores — batch-data-parallel (2 batch/core).

v7: every attention matmul is a full-rate K=128 fp8 DoubleRow: q/k live in
zero-padded staging tiles (real rows at the head's partition offset, zero
rows + a zero second k-tile elsewhere) so the K=64-per-head contractions
stream at the double-pumped rate; the relative-position B-add is folded into
the A^T skew-transpose DoubleRows (lhsT pair [c1-block | identity], rhs pair
[identity | c2-block]); c2c opens each score PSUM group as its own DoubleRow.
Projections / P@V / FFN-W1 are fp8 DoubleRow with host-prequantized
per-output-column weights, W2 stays bf16.  All dequant+bias applies run on
the vector engine via tensor_scalar with AP scalars (scalar engine keeps only
exp / gelu / sqrt and band-edge copies).  hs/pos arrive partition-major from
the host; softmax normalization is in-loop (reciprocal + gpsimd
partition_broadcast); LN stats are fused into the Wo/W2 loops and each LN
apply overlaps the other token-half's matmuls; output leaves feature-major
bf16 and is transposed on the host.
"""

import os
import sys

sys.path.insert(0, "/opt/trn_rl_repo")

import numpy as np
import ml_dtypes

import concourse.bass as bass
import concourse.mybir as mybir
import concourse.tile as tile
from concourse import bacc
from concourse.bass_utils import run_bass_kernel_spmd
from concourse.masks import make_identity

F32 = mybir.dt.float32
BF16 = mybir.dt.bfloat16
F8 = mybir.dt.float8e4
ADD = mybir.AluOpType.add
MULT = mybir.AluOpType.mult
SUB = mybir.AluOpType.subtract
AF = mybir.ActivationFunctionType
DR = mybir.MatmulPerfMode.DoubleRow
F8NP = ml_dtypes.float8_e4m3
BFNP = ml_dtypes.bfloat16

B, S, H, NH, DH, P, I = 16, 512, 768, 12, 64, 512, 3072
NCORES = 8
BL = B // NCORES
T = BL * S
FC = H // 128
R2P = 2 * P
SCALE = 1.0 / float(np.sqrt(3.0 * DH))
EPS = 1e-7
BAND = 640
SV = 8.0

OFF = dict(bq=0, bk=6, bo=12, b2=18, ln1g=24, ln1b=30, ln2g=36, ln2b=42,
           b1=48, sq=72, sk=78, spk=84, spq=90, so=96, sw1=102, svdeq=126)


def skew_read_ap(dram_tile):
    flat = dram_tile.rearrange("a b -> (a b)")
    return bass.AP(flat.tensor, flat.offset + 511,
                   [[1023, 128], [1023 * 128, 4], [1, 512]])


def band_write_ap(dram_tile):
    flat = dram_tile.rearrange("a b -> (a b)")
    return bass.AP(flat.tensor, flat.offset + 384,
                   [[1024, 128], [1024 * 128 - 128, 4], [1, BAND]])


def build_nc():
    nc = bacc.Bacc("TRN2", target_bir_lowering=False, debug=False,
                   enable_asserts=False, num_devices=NCORES)

    hsT_d = nc.dram_tensor("hsT", [128, FC * T], BF16, kind="ExternalInput").ap()
    hs8_d = nc.dram_tensor("hs8", [128, FC * T], F8, kind="ExternalInput").ap()
    pos8_d = nc.dram_tensor("pos8", [128, FC * R2P], F8, kind="ExternalInput").ap()
    wimg_d = {}
    for nm in ["wq8i", "wk8i", "wpk8i", "wpq8i", "wo8i"]:
        wimg_d[nm] = nc.dram_tensor(nm, [128, FC, 3, 2, 128], F8,
                                    kind="ExternalInput").ap()
    wv8_d = nc.dram_tensor("wv8i", [128, 3, 2, 2, 384], F8,
                           kind="ExternalInput").ap()
    w1_d = nc.dram_tensor("w1i", [128, 24, 3, 2, 128], F8,
                          kind="ExternalInput").ap()
    w2_d = nc.dram_tensor("w2i", [128, FC, 24, 128], BF16,
                          kind="ExternalInput").ap()
    aux_d = nc.dram_tensor("aux", [128, 128], F32, kind="ExternalInput").ap()
    out_d = nc.dram_tensor("out", [H, T], BF16, kind="ExternalOutput").ap()
    outv = out_d.rearrange("(c p) t -> p c t", p=128)

    from contextlib import ExitStack
    with tile.TileContext(nc) as tc, ExitStack() as ctx:
        const = ctx.enter_context(tc.tile_pool(name="const", bufs=1))
        res = ctx.enter_context(tc.tile_pool(name="res", bufs=1))
        wpool = ctx.enter_context(tc.tile_pool(name="wpool", bufs=2))
        w2pool = ctx.enter_context(tc.tile_pool(name="w2pool", bufs=4))
        work = ctx.enter_context(tc.tile_pool(name="work", bufs=2))
        lnrow = ctx.enter_context(tc.tile_pool(name="lnrow", bufs=1))
        stgp = ctx.enter_context(tc.tile_pool(name="stgp", bufs=2))
        psA = ctx.enter_context(tc.tile_pool(name="psA", bufs=2, space="PSUM"))
        psS = ctx.enter_context(tc.tile_pool(name="psS", bufs=2, space="PSUM"))
        psC = ctx.enter_context(tc.tile_pool(name="psC", bufs=2, space="PSUM"))
        psP = ctx.enter_context(tc.tile_pool(name="psP", bufs=2, space="PSUM"))
        dram = ctx.enter_context(tc.tile_pool(name="dram", bufs=4, space="DRAM"))

        # ---------------- constants ----------------
        identb = const.tile([128, 128], BF16, tag="idb")
        make_identity(nc, identb)
        ident8 = const.tile([128, 128], F8, tag="id8")
        nc.vector.tensor_copy(ident8, identb)
        ones_col_b = const.tile([128, 1], BF16, tag="ocb")
        nc.gpsimd.memset(ones_col_b, 1.0)
        ones_r128b = const.tile([1, 128], BF16, tag="o128")
        nc.gpsimd.memset(ones_r128b, 1.0)
        eps_t = const.tile([1, 1], F32, tag="eps")
        nc.gpsimd.memset(eps_t, EPS)
        aux = const.tile([128, 128], F32, tag="aux")
        nc.scalar.dma_start(aux, aux_d)

        def ax(name, i):
            o = OFF[name] + i
            return aux[:, o:o + 1]

        # ---------------- residents ----------------
        trunkA = res.tile([128, FC, T], BF16, tag="trunkA")
        trunk8 = res.tile([128, FC, T], F8, tag="t8")
        pos8sb = res.tile([128, FC, R2P], F8, tag="p8")
        arena = res.tile([128, 24, T], BF16, tag="arena")
        qT = arena[:, 0:6, :]
        kT = arena[:, 6:12, :]
        g1 = arena
        pos28 = res.tile([128, 13, R2P], F8, tag="pos28")   # row 12 = pad
        p28f = pos28.rearrange("p r u -> p (r u)")
        v65 = res.tile([128, 8, NH, 68], F8, tag="v65")
        ctx8 = res.tile([128, FC, T], F8, tag="t8")
        trunkB = res.tile([128, FC, T], BF16, tag="trunkB")
        trunkB8 = res.tile([128, FC, R2P], F8, tag="p8")
        yout = res.tile([128, FC, T], BF16, tag="trunkA")

        nc.sync.dma_start(trunkA.rearrange("p a b -> p (a b)"), hsT_d)
        nc.sync.dma_start(trunk8.rearrange("p a b -> p (a b)"), hs8_d)
        nc.sync.dma_start(pos8sb.rearrange("p a b -> p (a b)"), pos8_d)

        # attention staging
        ABs = [res.tile([128, 4, 2, 512], F8, tag=f"AB{i}", name=f"AB{i}")
               for i in range(3)]
        C2s = [res.tile([128, 5, 512], F8, tag=f"C2{i}", name=f"C2{i}")
               for i in range(3)]
        QBs = [res.tile([128, 2, 512], F8, tag=f"QB{i}", name=f"QB{i}")
               for i in range(6)]
        QB2s = [res.tile([128, 4, 2, 128], F8, tag=f"QB2{i}", name=f"QB2{i}")
                for i in range(6)]
        KB2s = [res.tile([128, 4, 2, 128], F8, tag=f"KB2{i}", name=f"KB2{i}")
                for i in range(6)]
        for i in range(3):
            for ic in range(4):
                for blk in range(4):
                    nc.vector.tensor_copy(
                        ABs[i][:, ic, 1, blk * 128:(blk + 1) * 128], ident8)
            for blk in range(4):
                nc.vector.tensor_copy(C2s[i][:, 0, blk * 128:(blk + 1) * 128],
                                      ident8)
        for i in range(6):
            nc.gpsimd.memset(QBs[i], 0.0)
            nc.gpsimd.memset(QB2s[i], 0.0)
            nc.gpsimd.memset(KB2s[i], 0.0)
        nc.gpsimd.memset(pos28[:, 12, :], 0.0)   # junk-pair spill row

        # ---------------- projections (fp8 DoubleRow) ----------------
        def projDR(wd, rhs8, dst_fn, s_name, b_name):
            wsb = wpool.tile([128, FC, 3, 2, 128], F8, tag="w8")
            nc.sync.dma_start(wsb, wd)
            for ofc in range(FC):
                for tt in range(2):
                    sl = slice(tt * 512, (tt + 1) * 512)
                    pool, ptag = (psP, "P") if (ofc * 2 + tt) % 2 == 0 \
                        else (psA, "A")
                    acc = pool.tile([128, 512], F32, tag=ptag)
                    for p in range(3):
                        nc.tensor.matmul(acc, wsb[:, ofc, p, :, :],
                                         rhs8[:, 2 * p:2 * p + 2, sl],
                                         start=(p == 0), stop=(p == 2),
                                         perf_mode=DR)
                    if b_name:
                        # scalar engine is idle during projections
                        nc.scalar.activation(dst_fn(ofc, sl), acc,
                                             AF.Identity,
                                             bias=ax(b_name, ofc),
                                             scale=ax(s_name, ofc))
                    else:
                        nc.vector.tensor_scalar_mul(dst_fn(ofc, sl), acc,
                                                    ax(s_name, ofc))

        projDR(wimg_d["wq8i"], trunk8, lambda o, sl: qT[:, o, sl], "sq", "bq")
        projDR(wimg_d["wk8i"], trunk8, lambda o, sl: kT[:, o, sl], "sk", "bk")
        projDR(wimg_d["wpk8i"], pos8sb, lambda o, sl: pos28[:, o, sl],
               "spk", None)
        projDR(wimg_d["wpq8i"], pos8sb, lambda o, sl: pos28[:, 6 + o, sl],
               "spq", None)

        # v: token-major into v65 with fused ones column (= SV)
        nc.gpsimd.memset(v65, SV)
        wv = wpool.tile([128, 3, 2, 2, 384], F8, tag="w8")
        nc.sync.dma_start(wv, wv8_d)
        for tcx in range(8):
            for half in range(2):
                pool, ptag = (psP, "P") if (tcx * 2 + half) % 2 == 0 \
                    else (psA, "A")
                acc = pool.tile([128, 512], F32, tag=ptag)
                for p in range(3):
                    nc.tensor.matmul(acc[:, 0:384],
                                     trunk8[:, 2 * p:2 * p + 2,
                                            tcx * 128:(tcx + 1) * 128],
                                     wv[:, p, :, half, :],
                                     start=(p == 0), stop=(p == 2),
                                     perf_mode=DR)
                dstv = v65[:, tcx, half * 6:(half + 1) * 6, 0:64]
                src = acc[:, 0:384].rearrange("p (a b) -> p a b", b=64)
                nc.vector.tensor_scalar_mul(dstv, src, ax("svdeq", 0))

        # ---------------- attention ----------------
        def pos_pair(row, off, n):
            # second k-tile = next row (junk, killed by zero lhsT rows)
            base = row * R2P + off
            return bass.AP(p28f.tensor, p28f.offset + base,
                           [p28f.ap[0], [R2P, 2], [1, n]])

        def ab_produce(b, h, slot6, slot3):
            fch = h // 2
            p0 = (h % 2) * 64
            bi = b * 512
            QB = QBs[slot6]
            QB2 = QB2s[slot6]
            KB2 = KB2s[slot6]
            qsl = qT[p0:p0 + 64, fch, bi:bi + 512]
            ksl = kT[p0:p0 + 64, fch, bi:bi + 512]
            nc.vector.tensor_copy(QB[p0:p0 + 64, 0, :], qsl)
            nc.vector.tensor_copy(QB2[p0:p0 + 64, :, 0, :],
                                  qsl.rearrange("p (a b) -> p a b", b=128))
            nc.vector.tensor_copy(KB2[p0:p0 + 64, :, 0, :],
                                  ksl.rearrange("p (a b) -> p a b", b=128))

            a_dram = dram.tile([512, R2P], F8, tag="Ad")
            b_dram = dram.tile([512, R2P], F8, tag="Bd")
            for mi, (src, prow, dst) in enumerate(
                    ((QB2, fch, a_dram), (KB2, 6 + fch, b_dram))):
                stg = stgp.tile([128, 4, BAND], F8, tag="stg")
                ed = psP.tile([128, 512], F32, tag="P")
                for c in range(4):
                    w0 = 384 - 128 * c
                    acc = psA.tile([128, 512], F32, tag="A")
                    nc.tensor.matmul(acc, src[:, c, :, :],
                                     pos_pair(prow, w0, 512),
                                     start=True, stop=True, perf_mode=DR)
                    nc.tensor.matmul(ed[:, c * 128:(c + 1) * 128],
                                     src[:, c, :, :],
                                     pos_pair(prow, w0 + 512, 128),
                                     start=True, stop=True, perf_mode=DR,
                                     skip_group_check=True)
                    if (mi + c) % 2 == 0:
                        nc.vector.tensor_copy(stg[:, c, 0:512], acc)
                    else:
                        nc.scalar.copy(stg[:, c, 0:512], acc)
                edv = ed.rearrange("p (a b) -> p a b", b=128)
                if mi == 0:
                    nc.scalar.copy(stg[:, :, 512:640], edv)
                else:
                    nc.vector.tensor_copy(stg[:, :, 512:640], edv)
                nc.sync.dma_start(band_write_ap(dst), stg)

            AB = ABs[slot3]
            nc.sync.dma_start(AB[:, :, 0, :], skew_read_ap(a_dram))
            C2 = C2s[slot3]
            nc.sync.dma_start(C2[:, 1:5, :], skew_read_ap(b_dram))
            return (b, h, slot6, slot3)

        def emit_tail(tail):
            if tail is None:
                return
            ctxden, tcbase, h, prb1, p0, fch, bi = tail
            nc.tensor.matmul(ctxden, v65[:, tcbase:tcbase + 2, h, 0:66], prb1,
                             start=False, stop=True, perf_mode=DR,
                             skip_group_check=True)
            lnt = work.tile([1, 512], BF16, tag="lnt")
            nc.scalar.activation(lnt, ctxden[64:65, :], AF.Ln, bias=0.0,
                                 scale=1.0)
            rec = work.tile([1, 512], BF16, tag="rec")
            nc.scalar.activation(rec, lnt, AF.Exp, bias=0.0, scale=-1.0)
            recb = work.tile([64, 512], BF16, tag="recb")
            nc.gpsimd.partition_broadcast(recb, rec)
            nc.vector.tensor_tensor(ctx8[p0:p0 + 64, fch, bi:bi + 512],
                                    ctxden[0:64, :], recb, MULT)

        def score_phase(b, h, slot6, slot3, tail):
            fch = h // 2
            p0 = (h % 2) * 64
            bi = b * 512
            AB = ABs[slot3]
            C2 = C2s[slot3]
            QB = QBs[slot6]
            KB2 = KB2s[slot6]
            emit_tail(tail)

            def do_jc(jc, prb, t):
                sc = psS.tile([128, 512], F32, tag="S")
                nc.tensor.matmul(sc, KB2[:, jc, :, :], QB,
                                 start=True, stop=False, perf_mode=DR)
                for ic in range(4):
                    rhs = bass.AP(C2.tensor, C2.offset + ic * 128,
                                  [C2.ap[0], [(1 + jc) * 512, 2], [1, 128]])
                    nc.tensor.matmul(sc[:, ic * 128:(ic + 1) * 128],
                                     AB[:, ic, :, jc * 128:(jc + 1) * 128],
                                     rhs, start=False, stop=(ic == 3),
                                     perf_mode=DR, skip_group_check=True)
                nc.scalar.activation(prb[:, t, :], sc, AF.Exp, bias=0.0,
                                     scale=SCALE)

            ctxden = psC.tile([66, 512], F32, tag="C")
            prb0 = work.tile([128, 2, 512], F8, tag="prb")
            prb1 = work.tile([128, 2, 512], F8, tag="prb")
            do_jc(0, prb0, 0)
            do_jc(1, prb0, 1)
            do_jc(2, prb1, 0)
            nc.tensor.matmul(ctxden, v65[:, b * 4:b * 4 + 2, h, 0:66], prb0,
                             start=True, stop=False, perf_mode=DR,
                             skip_group_check=True)
            do_jc(3, prb1, 1)
            return (ctxden, b * 4 + 2, h, prb1, p0, fch, bi)

        order = [(b, h) for b in range(BL) for h in range(NH)]
        pend = []
        tail = None
        for idx in range(len(order) + 2):
            if idx < len(order):
                pend.append(ab_produce(*order[idx], slot6=idx % 6,
                                       slot3=idx % 3))
            if idx >= 2:
                tail = score_phase(*pend.pop(0), tail)
        emit_tail(tail)

        # ---------------- shared LN finalize+apply ----------------
        def ln_finalize_apply(x, y, ssum, ssq, gname, bname, tt,
                              y8=None, store=False):
            sl = slice(tt * 512, (tt + 1) * 512)
            mu = lnrow.tile([1, 512], F32, tag="mu")
            nc.vector.tensor_scalar_mul(mu, ssum[0:1, :], 1.0 / H)
            msq = lnrow.tile([1, 512], F32, tag="msq")
            nc.vector.tensor_scalar_mul(msq, ssq[0:1, :], 1.0 / H)
            var = lnrow.tile([1, 512], F32, tag="var")
            nc.vector.tensor_tensor(var, mu, mu, MULT)
            nc.vector.tensor_tensor(var, msq, var, SUB)
            sd = lnrow.tile([1, 512], F32, tag="sd")
            nc.scalar.activation(sd, var, AF.Sqrt, bias=eps_t, scale=1.0)
            rstd = lnrow.tile([1, 512], BF16, tag="rstd")
            with nc.allow_low_precision(reason="ln rstd bf16"):
                nc.vector.reciprocal(rstd, sd)
            mur = lnrow.tile([1, 512], BF16, tag="mur")
            nc.vector.tensor_tensor(mur, mu, rstd, MULT)
            pb = psA.tile([128, 512], F32, tag="A")
            nc.tensor.matmul(pb, ones_r128b, rstd, start=True, stop=True)
            pb2 = psA.tile([128, 512], F32, tag="A")
            nc.tensor.matmul(pb2, ones_r128b, mur, start=True, stop=True)
            for fc in range(FC):
                t1 = work.tile([128, 512], F32, tag="tmp")
                nc.vector.tensor_tensor(t1, x[:, fc, sl], pb, MULT)
                nc.vector.tensor_tensor(t1, t1, pb2, SUB)
                nc.scalar.activation(y[:, fc, sl], t1, AF.Identity,
                                     bias=ax(bname, fc), scale=ax(gname, fc))
                if y8 is not None:
                    nc.vector.tensor_copy(y8[:, fc, sl], y[:, fc, sl])
                if store:
                    nc.sync.dma_start(outv[:, fc, sl], y[:, fc, sl])

        # ---------------- Wo + residual + LN1 (per token-half) ------------
        wo = wpool.tile([128, FC, 3, 2, 128], F8, tag="w8")
        nc.sync.dma_start(wo, wimg_d["wo8i"])
        w1sbs = []
        for tt in range(2):
            sl = slice(tt * 512, (tt + 1) * 512)
            spool, stag = (psA, "A") if tt == 0 else (psS, "S")
            ssum = spool.tile([128, 512], F32, tag=stag, name=f"ssum1{tt}")
            ssq = spool.tile([128, 512], F32, tag=stag, name=f"ssq1{tt}")
            for ofc in range(FC):
                acc = psP.tile([128, 512], F32, tag="P")
                for p in range(3):
                    nc.tensor.matmul(acc, wo[:, ofc, p, :, :],
                                     ctx8[:, 2 * p:2 * p + 2, sl],
                                     start=(p == 0), stop=(p == 2),
                                     perf_mode=DR)
                tmp = work.tile([128, 512], F32, tag="tmp")
                nc.vector.tensor_scalar(tmp, acc, ax("so", ofc),
                                        ax("bo", ofc), MULT, ADD)
                nc.vector.tensor_tensor(trunkA[:, ofc, sl],
                                        trunkA[:, ofc, sl], tmp, ADD)
                nc.tensor.matmul(ssum[0:1, :], ones_col_b, trunkA[:, ofc, sl],
                                 start=(ofc == 0), stop=(ofc == 5),
                                 skip_group_check=True)
                sq = work.tile([128, 512], BF16, tag="sq")
                nc.vector.tensor_tensor(sq, trunkA[:, ofc, sl],
                                        trunkA[:, ofc, sl], MULT)
                nc.tensor.matmul(ssq[0:1, :], ones_col_b, sq,
                                 start=(ofc == 0), stop=(ofc == 5),
                                 skip_group_check=True)
            if tt == 0:
                w1sbs.append(wpool.tile([128, 6, 3, 2, 128], F8, tag="w8",
                                        name="w1sb0"))
                nc.sync.dma_start(w1sbs[0], w1_d[:, 0:6])
            ln_finalize_apply(trunkA, trunkB, ssum, ssq, "ln1g", "ln1b", tt,
                              y8=trunkB8)

        # ---------------- FFN ----------------
        for wc in range(4):
            if wc > 0:
                w1sbs.append(wpool.tile([128, 6, 3, 2, 128], F8, tag="w8",
                                        name=f"w1sb{wc}"))
                nc.sync.dma_start(w1sbs[wc], w1_d[:, wc * 6:(wc + 1) * 6])
            w1sb = w1sbs[wc]
            for ol in range(6):
                ofc = wc * 6 + ol
                for tt in range(2):
                    sl = slice(tt * 512, (tt + 1) * 512)
                    pool, ptag = (psP, "P") if (ofc * 2 + tt) % 2 == 0 \
                        else (psA, "A")
                    acc = pool.tile([128, 512], F32, tag=ptag)
                    for p in range(3):
                        nc.tensor.matmul(acc, w1sb[:, ol, p, :, :],
                                         trunkB8[:, 2 * p:2 * p + 2, sl],
                                         start=(p == 0), stop=(p == 2),
                                         perf_mode=DR)
                    nc.scalar.activation(g1[:, ofc, sl], acc, AF.Gelu,
                                         bias=ax("b1", ofc),
                                         scale=ax("sw1", ofc))

        w2sbs = []
        for ofc in range(4):
            w2sbs.append(w2pool.tile([128, 24, 128], BF16, tag="w2",
                                     name=f"w2sb{ofc}"))
            nc.sync.dma_start(w2sbs[ofc], w2_d[:, ofc])
        stats2 = {}
        for tt in range(2):
            spool, stag = (psA, "A") if tt == 0 else (psS, "S")
            stats2[tt] = (
                spool.tile([128, 512], F32, tag=stag, name=f"ssum2{tt}"),
                spool.tile([128, 512], F32, tag=stag, name=f"ssq2{tt}"))
        for g in range(2):
            if g == 1:
                for ofc in (4, 5):
                    w2sbs.append(w2pool.tile([128, 24, 128], BF16, tag="w2",
                                             name=f"w2sb{ofc}"))
                    nc.sync.dma_start(w2sbs[ofc], w2_d[:, ofc])
            for tt in range(2):
                sl = slice(tt * 512, (tt + 1) * 512)
                ssum, ssq = stats2[tt]
                for j in range(3):
                    ofc = 3 * g + j
                    acc = psP.tile([128, 512], F32, tag="P")
                    for kc in range(24):
                        nc.tensor.matmul(acc, w2sbs[ofc][:, kc, :],
                                         g1[:, kc, sl],
                                         start=(kc == 0), stop=(kc == 23),
                                         skip_group_check=True)
                    nc.vector.scalar_tensor_tensor(trunkB[:, ofc, sl], acc,
                                                   ax("b2", ofc),
                                                   trunkB[:, ofc, sl],
                                                   ADD, ADD)
                    nc.tensor.matmul(ssum[0:1, :], ones_col_b,
                                     trunkB[:, ofc, sl],
                                     start=(ofc == 0), stop=(ofc == 5),
                                     skip_group_check=True)
                    sq = work.tile([128, 512], BF16, tag="sq")
                    nc.vector.tensor_tensor(sq, trunkB[:, ofc, sl],
                                            trunkB[:, ofc, sl], MULT)
                    nc.tensor.matmul(ssq[0:1, :], ones_col_b, sq,
                                     start=(ofc == 0), stop=(ofc == 5),
                                     skip_group_check=True)
                if g == 1:
                    ssum_t, ssq_t = stats2[tt]
                    ln_finalize_apply(trunkB, yout, ssum_t, ssq_t,
                                      "ln2g", "ln2b", tt, store=True)

    nc.finalize()
    return nc


# ---------------- host side ----------------

def _qcol(W):
    absmax = np.maximum(np.abs(W).max(axis=0), 1e-20)
    s = 224.0 / absmax
    W8 = (W * s[None, :]).astype(F8NP)
    return W8, (1.0 / s).astype(np.float32)


def _img6(W8):
    return np.ascontiguousarray(
        W8.reshape(3, 2, 128, 6, 128).transpose(2, 3, 0, 1, 4))


def _pm(x):
    """[768, N] -> [128, 6*N] partition-major image (f = c*128 + p)."""
    n = x.shape[1]
    return np.ascontiguousarray(
        x.reshape(6, 128, n).transpose(1, 0, 2).reshape(128, 6 * n))


def _prep_shared(inputs):
    pos = np.asarray(inputs["pos_emb"], np.float32)
    posT = np.ascontiguousarray(pos[::-1].T).astype(BFNP)
    shared = {"pos8": _pm(posT.astype(F8NP))}

    aux = np.zeros((128, 128), np.float32)

    def put6(name, vec):
        aux[:, OFF[name]:OFF[name] + 6] = np.asarray(
            vec, np.float32).reshape(6, 128).T

    def put24(name, vec):
        aux[:, OFF[name]:OFF[name] + 24] = np.asarray(
            vec, np.float32).reshape(24, 128).T

    for nm, key in [("bq", "bq"), ("bk", "bk"), ("bo", "bo"), ("b2", "b2"),
                    ("ln1g", "ln1_g"), ("ln1b", "ln1_b"),
                    ("ln2g", "ln2_g"), ("ln2b", "ln2_b")]:
        put6(nm, inputs[key])
    put24("b1", inputs["b1"])

    for wkey, iname, sname in [("Wq", "wq8i", "sq"), ("Wk", "wk8i", "sk"),
                               ("Wpk", "wpk8i", "spk"), ("Wpq", "wpq8i", "spq"),
                               ("Wo", "wo8i", "so")]:
        W8, dq = _qcol(np.asarray(inputs[wkey], np.float32))
        shared[iname] = _img6(W8)
        put6(sname, dq)

    Wv = np.asarray(inputs["Wv"], np.float32)
    sv = 224.0 / max(np.abs(Wv).max(), 1e-20)
    Wv8 = (Wv * sv).astype(F8NP)
    shared["wv8i"] = np.ascontiguousarray(
        Wv8.reshape(3, 2, 128, 2, 384).transpose(2, 0, 1, 3, 4))
    aux[:, OFF["svdeq"]] = SV / sv

    W18, dq1 = _qcol(np.asarray(inputs["W1"], np.float32))
    shared["w1i"] = np.ascontiguousarray(
        W18.reshape(3, 2, 128, 24, 128).transpose(2, 3, 0, 1, 4))
    put24("sw1", dq1)

    W2b = np.asarray(inputs["W2"], np.float32).astype(BFNP)
    shared["w2i"] = np.ascontiguousarray(
        W2b.reshape(24, 128, 6, 128).transpose(1, 2, 0, 3))

    shared["aux"] = aux
    return shared


_CACHE = {}


def _install_ntff_hook():
    import types
    try:
        import antenv.axon_hooks  # noqa: F401
        return
    except ImportError:
        pass
    try:
        from trn_agent_boot.trn_boot import _ntff_profile_via_ctypes
        hook = _ntff_profile_via_ctypes("/opt/axon/libaxon_pjrt.so")
        if hook is None:
            return
        mod = types.ModuleType("antenv.axon_hooks")
        mod._hook = hook
        mod.get_axon_ntff_profile_hook = lambda: mod._hook
        mod.set_axon_ntff_profile_hook = lambda h: setattr(mod, "_hook", h)
        sys.modules["antenv.axon_hooks"] = mod
        import antenv
        antenv.axon_hooks = mod
    except Exception as e:  # pragma: no cover
        print("ntff hook install failed:", e)


def kernel(**inputs):
    if "nc" not in _CACHE:
        _CACHE["nc"] = build_nc()
    nc = _CACHE["nc"]

    shared = _prep_shared(inputs)
    hs = np.asarray(inputs["hidden_states"], np.float32)

    in_maps = []
    for c in range(NCORES):
        m = dict(shared)
        hsT = np.ascontiguousarray(
            hs[c * BL:(c + 1) * BL].reshape(T, H).T).astype(BFNP)
        m["hsT"] = _pm(hsT)
        m["hs8"] = _pm(hsT.astype(F8NP))
        in_maps.append(m)

    trace = bool(int(os.environ.get("KTRACE", "0")))
    if trace:
        _install_ntff_hook()
    res = run_bass_kernel_spmd(nc, in_maps, core_ids=list(range(NCORES)),
                               trace=trace)
    _CACHE["last_results"] = res
    outs = []
    for r in res.results:
        o = np.asarray(r["out"]).astype(np.float32)
        outs.append(o.T.reshape(BL, S, H))
    return np.concatenate(outs, axis=0)


# revision 33
# speedup vs baseline: 1.1232x; 1.1232x over previous
"""DeBERTa layer on 8 trn2 NeuronCores — batch-data-parallel (2 batch/core).

v7: every attention matmul is a full-rate K=128 fp8 DoubleRow: q/k live in
zero-padded staging tiles (real rows at the head's partition offset, zero
rows + a zero second k-tile elsewhere) so the K=64-per-head contractions
stream at the double-pumped rate; the relative-position B-add is folded into
the A^T skew-transpose DoubleRows (lhsT pair [c1-block | identity], rhs pair
[identity | c2-block]); c2c opens each score PSUM group as its own DoubleRow.
Projections / P@V / FFN-W1 are fp8 DoubleRow with host-prequantized
per-output-column weights, W2 stays bf16.  All dequant+bias applies run on
the vector engine via tensor_scalar with AP scalars (scalar engine keeps only
exp / gelu / sqrt and band-edge copies).  hs/pos arrive partition-major from
the host; softmax normalization is in-loop (reciprocal + gpsimd
partition_broadcast); LN stats are fused into the Wo/W2 loops and each LN
apply overlaps the other token-half's matmuls; output leaves feature-major
bf16 and is transposed on the host.
"""

import os
import sys

sys.path.insert(0, "/opt/trn_rl_repo")

import numpy as np
import ml_dtypes

import concourse.bass as bass
import concourse.mybir as mybir
import concourse.tile as tile
from concourse import bacc
from concourse.bass_utils import run_bass_kernel_spmd
from concourse.masks import make_identity

F32 = mybir.dt.float32
BF16 = mybir.dt.bfloat16
F8 = mybir.dt.float8e4
ADD = mybir.AluOpType.add
MULT = mybir.AluOpType.mult
SUB = mybir.AluOpType.subtract
AF = mybir.ActivationFunctionType
DR = mybir.MatmulPerfMode.DoubleRow
F8NP = ml_dtypes.float8_e4m3
BFNP = ml_dtypes.bfloat16

B, S, H, NH, DH, P, I = 16, 512, 768, 12, 64, 512, 3072
NCORES = 8
BL = B // NCORES
T = BL * S
FC = H // 128
R2P = 2 * P
SCALE = 1.0 / float(np.sqrt(3.0 * DH))
EPS = 1e-7
BAND = 640
SV = 8.0

OFF = dict(bq=0, bk=6, bo=12, b2=18, ln1g=24, ln1b=30, ln2g=36, ln2b=42,
           b1=48, sq=72, sk=78, spk=84, spq=90, so=96, sw1=102, svdeq=126)


def skew_read_ap(dram_tile):
    flat = dram_tile.rearrange("a b -> (a b)")
    return bass.AP(flat.tensor, flat.offset + 511,
                   [[1023, 128], [1023 * 128, 4], [1, 512]])


def band_write_ap(dram_tile):
    flat = dram_tile.rearrange("a b -> (a b)")
    return bass.AP(flat.tensor, flat.offset + 384,
                   [[1024, 128], [1024 * 128 - 128, 4], [1, BAND]])


def build_nc():
    nc = bacc.Bacc("TRN2", target_bir_lowering=False, debug=False,
                   enable_asserts=False, num_devices=NCORES)

    hsT_d = nc.dram_tensor("hsT", [128, FC * T], BF16, kind="ExternalInput").ap()
    hs8_d = nc.dram_tensor("hs8", [128, FC * T], F8, kind="ExternalInput").ap()
    pos8_d = nc.dram_tensor("pos8", [128, FC * R2P], F8, kind="ExternalInput").ap()
    wimg_d = {}
    for nm in ["wq8i", "wk8i", "wpk8i", "wpq8i", "wo8i"]:
        wimg_d[nm] = nc.dram_tensor(nm, [128, FC, 3, 2, 128], F8,
                                    kind="ExternalInput").ap()
    wv8_d = nc.dram_tensor("wv8i", [128, 3, 2, 2, 384], F8,
                           kind="ExternalInput").ap()
    w1_d = nc.dram_tensor("w1i", [128, 24, 3, 2, 128], F8,
                          kind="ExternalInput").ap()
    w2_d = nc.dram_tensor("w2i", [128, FC, 24, 128], BF16,
                          kind="ExternalInput").ap()
    aux_d = nc.dram_tensor("aux", [128, 128], F32, kind="ExternalInput").ap()
    out_d = nc.dram_tensor("out", [H, T], BF16, kind="ExternalOutput").ap()
    outv = out_d.rearrange("(c p) t -> p c t", p=128)

    from contextlib import ExitStack
    with tile.TileContext(nc) as tc, ExitStack() as ctx:
        const = ctx.enter_context(tc.tile_pool(name="const", bufs=1))
        res = ctx.enter_context(tc.tile_pool(name="res", bufs=1))
        wpool = ctx.enter_context(tc.tile_pool(name="wpool", bufs=2))
        w2pool = ctx.enter_context(tc.tile_pool(name="w2pool", bufs=4))
        work = ctx.enter_context(tc.tile_pool(name="work", bufs=2))
        lnrow = ctx.enter_context(tc.tile_pool(name="lnrow", bufs=1))
        stgp = ctx.enter_context(tc.tile_pool(name="stgp", bufs=2))
        psA = ctx.enter_context(tc.tile_pool(name="psA", bufs=2, space="PSUM"))
        psS = ctx.enter_context(tc.tile_pool(name="psS", bufs=2, space="PSUM"))
        psC = ctx.enter_context(tc.tile_pool(name="psC", bufs=2, space="PSUM"))
        psP = ctx.enter_context(tc.tile_pool(name="psP", bufs=2, space="PSUM"))
        dram = ctx.enter_context(tc.tile_pool(name="dram", bufs=4, space="DRAM"))

        # ---------------- constants ----------------
        identb = const.tile([128, 128], BF16, tag="idb")
        make_identity(nc, identb)
        ident8 = const.tile([128, 128], F8, tag="id8")
        nc.vector.tensor_copy(ident8, identb)
        ones_col_b = const.tile([128, 1], BF16, tag="ocb")
        nc.gpsimd.memset(ones_col_b, 1.0)
        ones_r128b = const.tile([1, 128], BF16, tag="o128")
        nc.gpsimd.memset(ones_r128b, 1.0)
        eps_t = const.tile([1, 1], F32, tag="eps")
        nc.gpsimd.memset(eps_t, EPS)
        aux = const.tile([128, 128], F32, tag="aux")
        nc.scalar.dma_start(aux, aux_d)

        def ax(name, i):
            o = OFF[name] + i
            return aux[:, o:o + 1]

        # ---------------- residents ----------------
        trunkA = res.tile([128, FC, T], BF16, tag="trunkA")
        trunk8 = res.tile([128, FC, T], F8, tag="t8")
        pos8sb = res.tile([128, FC, R2P], F8, tag="p8")
        arena = res.tile([128, 24, T], BF16, tag="arena")
        qT = arena[:, 0:6, :]
        kT = arena[:, 6:12, :]
        g1 = arena
        pos28 = res.tile([128, 13, R2P], F8, tag="pos28")   # row 12 = pad
        p28f = pos28.rearrange("p r u -> p (r u)")
        v65 = res.tile([128, 8, NH, 68], F8, tag="v65")
        ctx8 = res.tile([128, FC, T], F8, tag="t8")
        trunkB = res.tile([128, FC, T], BF16, tag="trunkB")
        trunkB8 = res.tile([128, FC, R2P], F8, tag="p8")
        yout = res.tile([128, FC, T], BF16, tag="trunkA")

        # trunk8 first (feeds the first projection), bulky trunkA (only
        # needed at the Wo residual) and pos8 ride the scalar queue
        nc.sync.dma_start(trunk8.rearrange("p a b -> p (a b)"), hs8_d)
        nc.scalar.dma_start(pos8sb.rearrange("p a b -> p (a b)"), pos8_d)
        nc.scalar.dma_start(trunkA.rearrange("p a b -> p (a b)"), hsT_d)

        # attention staging (content initialized after the projections)
        ABs = [res.tile([128, 4, 2, 512], F8, tag=f"AB{i}", name=f"AB{i}")
               for i in range(3)]
        C2s = [res.tile([128, 5, 512], F8, tag=f"C2{i}", name=f"C2{i}")
               for i in range(3)]
        QBs = [res.tile([128, 2, 512], F8, tag=f"QB{i}", name=f"QB{i}")
               for i in range(6)]
        KBs = [res.tile([128, 2, 512], F8, tag=f"KB{i}", name=f"KB{i}")
               for i in range(6)]
        nc.gpsimd.memset(pos28[:, 12, :], 0.0)   # junk-pair spill row

        # ---------------- projections (fp8 DoubleRow) ----------------
        def projDR(wd, rhs8, dst_fn, s_name, b_name):
            wsb = wpool.tile([128, FC, 3, 2, 128], F8, tag="w8")
            nc.sync.dma_start(wsb, wd)
            for ofc in range(FC):
                for tt in range(2):
                    sl = slice(tt * 512, (tt + 1) * 512)
                    pool, ptag = (psP, "P") if (ofc * 2 + tt) % 2 == 0 \
                        else (psA, "A")
                    acc = pool.tile([128, 512], F32, tag=ptag)
                    for p in range(3):
                        nc.tensor.matmul(acc, wsb[:, ofc, p, :, :],
                                         rhs8[:, 2 * p:2 * p + 2, sl],
                                         start=(p == 0), stop=(p == 2),
                                         perf_mode=DR)
                    if b_name:
                        # scalar engine is idle during projections
                        nc.scalar.activation(dst_fn(ofc, sl), acc,
                                             AF.Identity,
                                             bias=ax(b_name, ofc),
                                             scale=ax(s_name, ofc))
                    else:
                        nc.vector.tensor_scalar_mul(dst_fn(ofc, sl), acc,
                                                    ax(s_name, ofc))

        projDR(wimg_d["wq8i"], trunk8, lambda o, sl: qT[:, o, sl], "sq", "bq")
        projDR(wimg_d["wk8i"], trunk8, lambda o, sl: kT[:, o, sl], "sk", "bk")
        projDR(wimg_d["wpk8i"], pos8sb, lambda o, sl: pos28[:, o, sl],
               "spk", None)
        projDR(wimg_d["wpq8i"], pos8sb, lambda o, sl: pos28[:, 6 + o, sl],
               "spq", None)

        # v: token-major into v65 with fused ones column (= SV)
        nc.gpsimd.memset(v65, SV)
        wv = wpool.tile([128, 3, 2, 2, 384], F8, tag="w8")
        nc.sync.dma_start(wv, wv8_d)
        for tcx in range(8):
            for half in range(2):
                pool, ptag = (psP, "P") if (tcx * 2 + half) % 2 == 0 \
                    else (psA, "A")
                acc = pool.tile([128, 512], F32, tag=ptag)
                for p in range(3):
                    nc.tensor.matmul(acc[:, 0:384],
                                     trunk8[:, 2 * p:2 * p + 2,
                                            tcx * 128:(tcx + 1) * 128],
                                     wv[:, p, :, half, :],
                                     start=(p == 0), stop=(p == 2),
                                     perf_mode=DR)
                dstv = v65[:, tcx, half * 6:(half + 1) * 6, 0:64]
                src = acc[:, 0:384].rearrange("p (a b) -> p a b", b=64)
                nc.scalar.activation(dstv, src, AF.Identity, bias=0.0,
                                     scale=ax("svdeq", 0))

        # staging-content init here so it overlaps the projection tail
        for i in range(3):
            for ic in range(4):
                for blk in range(4):
                    nc.vector.tensor_copy(
                        ABs[i][:, ic, 1, blk * 128:(blk + 1) * 128], ident8)
            for blk in range(4):
                nc.vector.tensor_copy(C2s[i][:, 0, blk * 128:(blk + 1) * 128],
                                      ident8)
        for i in range(6):
            nc.gpsimd.memset(QBs[i], 0.0)
            nc.gpsimd.memset(KBs[i], 0.0)

        # ---------------- attention ----------------
        def pos_pair(row, off, n):
            # second k-tile = next row (junk, killed by zero lhsT rows)
            base = row * R2P + off
            return bass.AP(p28f.tensor, p28f.offset + base,
                           [p28f.ap[0], [R2P, 2], [1, n]])

        def ab_produce(b, h, slot6, slot3):
            fch = h // 2
            p0 = (h % 2) * 64
            bi = b * 512
            QB = QBs[slot6]
            KB = KBs[slot6]
            nc.vector.tensor_copy(QB[p0:p0 + 64, 0, :],
                                  qT[p0:p0 + 64, fch, bi:bi + 512])
            nc.vector.tensor_copy(KB[p0:p0 + 64, 0, :],
                                  kT[p0:p0 + 64, fch, bi:bi + 512])

            a_dram = dram.tile([512, R2P], F8, tag="Ad")
            b_dram = dram.tile([512, R2P], F8, tag="Bd")
            for mi, (src, prow, dst) in enumerate(
                    ((QB, fch, a_dram), (KB, 6 + fch, b_dram))):
                stg = stgp.tile([128, 4, BAND], F8, tag="stg")
                ed = psP.tile([128, 512], F32, tag="P")
                for c in range(4):
                    w0 = 384 - 128 * c
                    acc = psA.tile([128, 512], F32, tag="A")
                    nc.tensor.matmul(acc, src[:, :, c * 128:(c + 1) * 128],
                                     pos_pair(prow, w0, 512),
                                     start=True, stop=True, perf_mode=DR)
                    nc.tensor.matmul(ed[:, c * 128:(c + 1) * 128],
                                     src[:, :, c * 128:(c + 1) * 128],
                                     pos_pair(prow, w0 + 512, 128),
                                     start=True, stop=True, perf_mode=DR,
                                     skip_group_check=True)
                    if (mi + c) % 2 == 0:
                        nc.vector.tensor_copy(stg[:, c, 0:512], acc)
                    else:
                        nc.scalar.copy(stg[:, c, 0:512], acc)
                edv = ed.rearrange("p (a b) -> p a b", b=128)
                if mi == 0:
                    nc.scalar.copy(stg[:, :, 512:640], edv)
                else:
                    nc.vector.tensor_copy(stg[:, :, 512:640], edv)
                nc.sync.dma_start(band_write_ap(dst), stg)

            AB = ABs[slot3]
            nc.sync.dma_start(AB[:, :, 0, :], skew_read_ap(a_dram))
            C2 = C2s[slot3]
            nc.sync.dma_start(C2[:, 1:5, :], skew_read_ap(b_dram))
            return (b, h, slot6, slot3)

        def emit_tail(tail):
            if tail is None:
                return
            ctxden, tcbase, h, prb1, p0, fch, bi = tail
            nc.tensor.matmul(ctxden, v65[:, tcbase:tcbase + 2, h, 0:66], prb1,
                             start=False, stop=True, perf_mode=DR,
                             skip_group_check=True)
            bh = ((tcbase - 2) // 4) * NH + h
            den_s = work.tile([1, 512], BF16, tag="rec")
            nc.vector.tensor_copy(den_s, ctxden[64:65, :])
            nc.scalar.dma_start(den24[bh:bh + 1, :], den_s)
            nc.vector.tensor_scalar_mul(ctx8[p0:p0 + 64, fch, bi:bi + 512],
                                        ctxden[0:64, :], 1.0 / 64.0)

        def score_phase(b, h, slot6, slot3, tail):
            fch = h // 2
            p0 = (h % 2) * 64
            bi = b * 512
            AB = ABs[slot3]
            C2 = C2s[slot3]
            QB = QBs[slot6]
            KB = KBs[slot6]
            emit_tail(tail)

            def do_jc(jc, prb, t):
                sc = psS.tile([128, 512], F32, tag="S")
                nc.tensor.matmul(sc, KB[:, :, jc * 128:(jc + 1) * 128], QB,
                                 start=True, stop=False, perf_mode=DR)
                for ic in range(4):
                    rhs = bass.AP(C2.tensor, C2.offset + ic * 128,
                                  [C2.ap[0], [(1 + jc) * 512, 2], [1, 128]])
                    nc.tensor.matmul(sc[:, ic * 128:(ic + 1) * 128],
                                     AB[:, ic, :, jc * 128:(jc + 1) * 128],
                                     rhs, start=False, stop=(ic == 3),
                                     perf_mode=DR, skip_group_check=True)
                nc.scalar.activation(prb[:, t, :], sc, AF.Exp, bias=0.0,
                                     scale=SCALE)

            ctxden = psC.tile([66, 512], F32, tag="C")
            prb0 = work.tile([128, 2, 512], F8, tag="prb")
            prb1 = work.tile([128, 2, 512], F8, tag="prb")
            do_jc(0, prb0, 0)
            do_jc(1, prb0, 1)
            do_jc(2, prb1, 0)
            nc.tensor.matmul(ctxden, v65[:, b * 4:b * 4 + 2, h, 0:66], prb0,
                             start=True, stop=False, perf_mode=DR,
                             skip_group_check=True)
            do_jc(3, prb1, 1)
            return (ctxden, b * 4 + 2, h, prb1, p0, fch, bi)

        den24 = lnrow.tile([24, 512], BF16, tag="den24")
        order = [(b, h) for b in range(BL) for h in range(NH)]
        pend = []
        tail = None
        for idx in range(len(order) + 2):
            if idx < len(order):
                pend.append(ab_produce(*order[idx], slot6=idx % 6,
                                       slot3=idx % 3))
            if idx >= 2:
                tail = score_phase(*pend.pop(0), tail)
        emit_tail(tail)

        # batched softmax normalization: one reciprocal, then per-head
        # broadcast+multiply (overlaps the Wo matmuls)
        recip24 = lnrow.tile([24, 512], BF16, tag="recip24")
        with nc.allow_low_precision(reason="softmax denom recip bf16"):
            nc.vector.reciprocal(recip24, den24)
        nc.vector.tensor_scalar_mul(recip24, recip24, 64.0)
        for b in range(BL):
            for h in range(NH):
                bh = b * NH + h
                fch = h // 2
                p0 = (h % 2) * 64
                bi = b * 512
                row = work.tile([1, 512], BF16, tag="rec")
                nc.scalar.dma_start(row, recip24[bh:bh + 1, :])
                recb = work.tile([128, 512], BF16, tag="recb")
                nc.gpsimd.partition_broadcast(recb, row)
                nc.vector.tensor_tensor(ctx8[p0:p0 + 64, fch, bi:bi + 512],
                                        ctx8[p0:p0 + 64, fch, bi:bi + 512],
                                        recb[p0:p0 + 64, :], MULT)

        # ---------------- shared LN finalize+apply ----------------
        def ln_finalize_apply(x, y, ssum, ssq, gname, bname, tt,
                              y8=None, store=False):
            sl = slice(tt * 512, (tt + 1) * 512)
            mu = lnrow.tile([1, 512], F32, tag="mu")
            nc.vector.tensor_scalar_mul(mu, ssum[0:1, :], 1.0 / H)
            msq = lnrow.tile([1, 512], F32, tag="msq")
            nc.vector.tensor_scalar_mul(msq, ssq[0:1, :], 1.0 / H)
            var = lnrow.tile([1, 512], F32, tag="var")
            nc.vector.tensor_tensor(var, mu, mu, MULT)
            nc.vector.tensor_tensor(var, msq, var, SUB)
            sd = lnrow.tile([1, 512], F32, tag="sd")
            nc.scalar.activation(sd, var, AF.Sqrt, bias=eps_t, scale=1.0)
            rstd = lnrow.tile([1, 512], BF16, tag="rstd")
            with nc.allow_low_precision(reason="ln rstd bf16"):
                nc.vector.reciprocal(rstd, sd)
            mur = lnrow.tile([1, 512], BF16, tag="mur")
            nc.vector.tensor_tensor(mur, mu, rstd, MULT)
            pb = psA.tile([128, 512], F32, tag="A")
            nc.tensor.matmul(pb, ones_r128b, rstd, start=True, stop=True)
            pb2 = psA.tile([128, 512], F32, tag="A")
            nc.tensor.matmul(pb2, ones_r128b, mur, start=True, stop=True)
            for fc in range(FC):
                t1 = work.tile([128, 512], F32, tag="tmp")
                nc.vector.tensor_tensor(t1, x[:, fc, sl], pb, MULT)
                nc.vector.tensor_tensor(t1, t1, pb2, SUB)
                nc.scalar.activation(y[:, fc, sl], t1, AF.Identity,
                                     bias=ax(bname, fc), scale=ax(gname, fc))
                if y8 is not None:
                    nc.vector.tensor_copy(y8[:, fc, sl], y[:, fc, sl])
                if store:
                    nc.sync.dma_start(outv[:, fc, sl], y[:, fc, sl])

        # ---------------- Wo + residual + LN1 (per token-half) ------------
        wo = wpool.tile([128, FC, 3, 2, 128], F8, tag="w8")
        nc.sync.dma_start(wo, wimg_d["wo8i"])
        w1sbs = []
        for tt in range(2):
            sl = slice(tt * 512, (tt + 1) * 512)
            spool, stag = (psA, "A") if tt == 0 else (psS, "S")
            ssum = spool.tile([128, 512], F32, tag=stag, name=f"ssum1{tt}")
            ssq = spool.tile([128, 512], F32, tag=stag, name=f"ssq1{tt}")
            for ofc in range(FC):
                acc = psP.tile([128, 512], F32, tag="P")
                for p in range(3):
                    nc.tensor.matmul(acc, wo[:, ofc, p, :, :],
                                     ctx8[:, 2 * p:2 * p + 2, sl],
                                     start=(p == 0), stop=(p == 2),
                                     perf_mode=DR)
                tmp = work.tile([128, 512], F32, tag="tmp")
                nc.vector.tensor_scalar(tmp, acc, ax("so", ofc),
                                        ax("bo", ofc), MULT, ADD)
                nc.vector.tensor_tensor(trunkA[:, ofc, sl],
                                        trunkA[:, ofc, sl], tmp, ADD)
                nc.tensor.matmul(ssum[0:1, :], ones_col_b, trunkA[:, ofc, sl],
                                 start=(ofc == 0), stop=(ofc == 5),
                                 skip_group_check=True)
                sq = work.tile([128, 512], BF16, tag="sq")
                nc.vector.tensor_tensor(sq, trunkA[:, ofc, sl],
                                        trunkA[:, ofc, sl], MULT)
                nc.tensor.matmul(ssq[0:1, :], ones_col_b, sq,
                                 start=(ofc == 0), stop=(ofc == 5),
                                 skip_group_check=True)
            if tt == 0:
                w1sbs.append(wpool.tile([128, 6, 3, 2, 128], F8, tag="w8",
                                        name="w1sb0"))
                nc.sync.dma_start(w1sbs[0], w1_d[:, 0:6])
            ln_finalize_apply(trunkA, trunkB, ssum, ssq, "ln1g", "ln1b", tt,
                              y8=trunkB8)

        # ---------------- FFN ----------------
        for wc in range(4):
            if wc > 0:
                w1sbs.append(wpool.tile([128, 6, 3, 2, 128], F8, tag="w8",
                                        name=f"w1sb{wc}"))
                nc.sync.dma_start(w1sbs[wc], w1_d[:, wc * 6:(wc + 1) * 6])
            w1sb = w1sbs[wc]
            for ol in range(6):
                ofc = wc * 6 + ol
                for tt in range(2):
                    sl = slice(tt * 512, (tt + 1) * 512)
                    pool, ptag = (psP, "P") if (ofc * 2 + tt) % 2 == 0 \
                        else (psA, "A")
                    acc = pool.tile([128, 512], F32, tag=ptag)
                    for p in range(3):
                        nc.tensor.matmul(acc, w1sb[:, ol, p, :, :],
                                         trunkB8[:, 2 * p:2 * p + 2, sl],
                                         start=(p == 0), stop=(p == 2),
                                         perf_mode=DR)
                    nc.scalar.activation(g1[:, ofc, sl], acc, AF.Gelu,
                                         bias=ax("b1", ofc),
                                         scale=ax("sw1", ofc))

        w2sbs = []
        for ofc in range(4):
            w2sbs.append(w2pool.tile([128, 24, 128], BF16, tag="w2",
                                     name=f"w2sb{ofc}"))
            nc.sync.dma_start(w2sbs[ofc], w2_d[:, ofc])
        stats2 = {}
        for tt in range(2):
            spool, stag = (psA, "A") if tt == 0 else (psS, "S")
            stats2[tt] = (
                spool.tile([128, 512], F32, tag=stag, name=f"ssum2{tt}"),
                spool.tile([128, 512], F32, tag=stag, name=f"ssq2{tt}"))
        for g in range(2):
            if g == 1:
                for ofc in (4, 5):
                    w2sbs.append(w2pool.tile([128, 24, 128], BF16, tag="w2",
                                             name=f"w2sb{ofc}"))
                    nc.sync.dma_start(w2sbs[ofc], w2_d[:, ofc])
            for tt in range(2):
                sl = slice(tt * 512, (tt + 1) * 512)
                ssum, ssq = stats2[tt]
                for j in range(3):
                    ofc = 3 * g + j
                    acc = psP.tile([128, 512], F32, tag="P")
                    for kc in range(24):
                        nc.tensor.matmul(acc, w2sbs[ofc][:, kc, :],
                                         g1[:, kc, sl],
                                         start=(kc == 0), stop=(kc == 23),
                                         skip_group_check=True)
                    nc.vector.scalar_tensor_tensor(trunkB[:, ofc, sl], acc,
                                                   ax("b2", ofc),
                                                   trunkB[:, ofc, sl],
                                                   ADD, ADD)
                    nc.tensor.matmul(ssum[0:1, :], ones_col_b,
                                     trunkB[:, ofc, sl],
                                     start=(ofc == 0), stop=(ofc == 5),
                                     skip_group_check=True)
                    sq = work.tile([128, 512], BF16, tag="sq")
                    nc.vector.tensor_tensor(sq, trunkB[:, ofc, sl],
                                            trunkB[:, ofc, sl], MULT)
                    nc.tensor.matmul(ssq[0:1, :], ones_col_b, sq,
                                     start=(ofc == 0), stop=(ofc == 5),
                                     skip_group_check=True)
                if g == 1:
                    ssum_t, ssq_t = stats2[tt]
                    ln_finalize_apply(trunkB, yout, ssum_t, ssq_t,
                                      "ln2g", "ln2b", tt, store=True)

    nc.finalize()
    return nc


# ---------------- host side ----------------

def _qcol(W):
    absmax = np.maximum(np.abs(W).max(axis=0), 1e-20)
    s = 224.0 / absmax
    W8 = (W * s[None, :]).astype(F8NP)
    return W8, (1.0 / s).astype(np.float32)


def _img6(W8):
    return np.ascontiguousarray(
        W8.reshape(3, 2, 128, 6, 128).transpose(2, 3, 0, 1, 4))


def _pm(x):
    """[768, N] -> [128, 6*N] partition-major image (f = c*128 + p)."""
    n = x.shape[1]
    return np.ascontiguousarray(
        x.reshape(6, 128, n).transpose(1, 0, 2).reshape(128, 6 * n))


def _prep_shared(inputs):
    pos = np.asarray(inputs["pos_emb"], np.float32)
    posT = np.ascontiguousarray(pos[::-1].T).astype(BFNP)
    shared = {"pos8": _pm(posT.astype(F8NP))}

    aux = np.zeros((128, 128), np.float32)

    def put6(name, vec):
        aux[:, OFF[name]:OFF[name] + 6] = np.asarray(
            vec, np.float32).reshape(6, 128).T

    def put24(name, vec):
        aux[:, OFF[name]:OFF[name] + 24] = np.asarray(
            vec, np.float32).reshape(24, 128).T

    for nm, key in [("bq", "bq"), ("bk", "bk"), ("bo", "bo"), ("b2", "b2"),
                    ("ln1g", "ln1_g"), ("ln1b", "ln1_b"),
                    ("ln2g", "ln2_g"), ("ln2b", "ln2_b")]:
        put6(nm, inputs[key])
    put24("b1", inputs["b1"])

    for wkey, iname, sname in [("Wq", "wq8i", "sq"), ("Wk", "wk8i", "sk"),
                               ("Wpk", "wpk8i", "spk"), ("Wpq", "wpq8i", "spq"),
                               ("Wo", "wo8i", "so")]:
        W8, dq = _qcol(np.asarray(inputs[wkey], np.float32))
        shared[iname] = _img6(W8)
        put6(sname, dq)

    Wv = np.asarray(inputs["Wv"], np.float32)
    sv = 224.0 / max(np.abs(Wv).max(), 1e-20)
    Wv8 = (Wv * sv).astype(F8NP)
    shared["wv8i"] = np.ascontiguousarray(
        Wv8.reshape(3, 2, 128, 2, 384).transpose(2, 0, 1, 3, 4))
    aux[:, OFF["svdeq"]] = SV / sv

    W18, dq1 = _qcol(np.asarray(inputs["W1"], np.float32))
    shared["w1i"] = np.ascontiguousarray(
        W18.reshape(3, 2, 128, 24, 128).transpose(2, 3, 0, 1, 4))
    put24("sw1", dq1)

    W2b = np.asarray(inputs["W2"], np.float32).astype(BFNP)
    shared["w2i"] = np.ascontiguousarray(
        W2b.reshape(24, 128, 6, 128).transpose(1, 2, 0, 3))

    shared["aux"] = aux
    return shared


_CACHE = {}


def _install_ntff_hook():
    import types
    try:
        import antenv.axon_hooks  # noqa: F401
        return
    except ImportError:
        pass
    try:
        from trn_agent_boot.trn_boot import _ntff_profile_via_ctypes
        hook = _ntff_profile_via_ctypes("/opt/axon/libaxon_pjrt.so")
        if hook is None:
            return
        mod = types.ModuleType("antenv.axon_hooks")
        mod._hook = hook
        mod.get_axon_ntff_profile_hook = lambda: mod._hook
        mod.set_axon_ntff_profile_hook = lambda h: setattr(mod, "_hook", h)
        sys.modules["antenv.axon_hooks"] = mod
        import antenv
        antenv.axon_hooks = mod
    except Exception as e:  # pragma: no cover
        print("ntff hook install failed:", e)


def kernel(**inputs):
    if "nc" not in _CACHE:
        _CACHE["nc"] = build_nc()
    nc = _CACHE["nc"]

    shared = _prep_shared(inputs)
    hs = np.asarray(inputs["hidden_states"], np.float32)

    in_maps = []
    for c in range(NCORES):
        m = dict(shared)
        hsT = np.ascontiguousarray(
            hs[c * BL:(c + 1) * BL].reshape(T, H).T).astype(BFNP)
        m["hsT"] = _pm(hsT)
        m["hs8"] = _pm(hsT.astype(F8NP))
        in_maps.append(m)

    trace = bool(int(os.environ.get("KTRACE", "0")))
    if trace:
        _install_ntff_hook()
    res = run_bass_kernel_spmd(nc, in_maps, core_ids=list(range(NCORES)),
                               trace=trace)
    _CACHE["last_results"] = res
    outs = []
    for r in res.results:
        o = np.asarray(r["out"]).astype(np.float32)
        outs.append(o.T.reshape(BL, S, H))
    return np.concatenate(outs, axis=0)
